# revision 1
# baseline (speedup 1.0000x reference)
"""2-layer GCN (GCNConv -> ReLU -> GCNConv -> log_softmax) on 8 TRN2 NeuronCores.

Strategy (graph/data parallel per the node-partition sharding):
- Nodes are sharded by destination range across the 8 cores; within each shard
  nodes are reordered by in-degree (descending) so that 128-node blocks have
  near-uniform degree, then each block's in-edge lists are padded to the
  block-max degree ("slots").
- Both GCN layers aggregate in 16-feature space (A_hat(yW2) = (A_hat y)W2, so
  the second layer's linear transform is applied after aggregation).
- Symmetric normalization folds into epilogues: table rows are pre-scaled by
  dis = deg^-1/2 and outputs post-scaled by dis; the self-loop is the node's
  own (pre-scaled) table row, used to initialize the block accumulator.
- Aggregation inner loop: per (block, slot), one indirect-DMA gather of 128
  table rows + one DVE add into the accumulator. Pad slots point at a
  guaranteed-zero table row. Layer tables are exchanged with AllGather.

kernel(**inputs) takes the full (unsharded) inputs and returns the full
[100000, 40] log-softmax output.
"""

import numpy as np
import concourse.bacc as bacc
import concourse.bass as bass
import concourse.mybir as mybir
from concourse.tile import TileContext
from concourse.masks import make_identity
from concourse.bass_utils import run_bass_kernel_spmd

F32 = mybir.dt.float32
I32 = mybir.dt.int32

# Problem shape (hardcoded per harness contract)
N_NODES = 100000
N_FEAT = 500
HID = 16
N_CLS = 40
N_CORES = 8


class _Cfg:
    def __init__(self, n_nodes, fin, hid, ncls, n_cores=8):
        self.N = n_nodes
        self.FIN = fin
        self.H = hid
        self.C = ncls
        self.NC = n_cores
        self.SHARD = n_nodes // n_cores
        assert self.SHARD * n_cores == n_nodes
        self.SHARD_PAD = ((self.SHARD + 127) // 128) * 128
        self.NB = self.SHARD_PAD // 128
        self.TROWS = n_cores * self.SHARD_PAD
        assert self.SHARD_PAD > self.SHARD, "need at least one pad row"
        self.ZPID = (n_cores - 1) * self.SHARD_PAD + self.SHARD
        self.KC = max(1, (fin + 127) // 128)
        assert fin % self.KC == 0
        self.CHUNK = fin // self.KC
        self.XS = 16  # blocks per xT supertile


def _preprocess(x, edge_index, cfg):
    """Host-side index work: permutation, slot arrays, per-core inputs."""
    N, NC, SP = cfg.N, cfg.NC, cfg.SHARD_PAD
    src = np.asarray(edge_index[0], dtype=np.int64)
    dst = np.asarray(edge_index[1], dtype=np.int64)
    deg = np.bincount(dst, minlength=N).astype(np.int64) + 1  # incl self-loop
    dis = (1.0 / np.sqrt(deg.astype(np.float64))).astype(np.float32)

    pid = np.empty(N, dtype=np.int64)
    perm_list = []
    for c in range(NC):
        nodes = np.arange(c * cfg.SHARD, (c + 1) * cfg.SHARD)
        order = np.argsort(-deg[nodes], kind="stable")
        local = nodes[order]
        perm_list.append(local)
        pid[local] = c * SP + np.arange(cfg.SHARD)

    src_pid = pid[src]
    dst_pid = pid[dst]

    core_of = dst_pid // SP
    S_per_core = np.zeros((NC, cfg.NB), dtype=np.int64)
    buckets = []
    for c in range(NC):
        m = core_of == c
        dl = dst_pid[m] - c * SP
        sp_ = src_pid[m]
        o = np.argsort(dl, kind="stable")
        dl, sp_ = dl[o], sp_[o]
        buckets.append((dl, sp_))
        cnt = np.bincount(dl, minlength=SP)
        S_per_core[c] = cnt.reshape(cfg.NB, 128).max(axis=1)
    S_list = S_per_core.max(axis=0).astype(np.int64)
    NI = int(S_list.sum())
    T_off = np.concatenate([[0], np.cumsum(S_list)])[:-1]

    offs = np.full((NC, 128, max(NI, 1)), cfg.ZPID, dtype=np.int32)
    for c in range(NC):
        dl, sp_ = buckets[c]
        cnt = np.bincount(dl, minlength=SP)
        starts = np.concatenate([[0], np.cumsum(cnt)])[:-1]
        b_arr = dl // 128
        p_arr = dl % 128
        s_arr = np.arange(dl.size) - starts[dl]
        t_arr = T_off[b_arr] + s_arr
        offs[c, p_arr, t_arr] = sp_.astype(np.int32)

    dis_pm = np.zeros((NC, 128, cfg.NB), dtype=np.float32)
    for c in range(NC):
        d = np.zeros(SP, dtype=np.float32)
        d[: cfg.SHARD] = dis[perm_list[c]]
        dis_pm[c] = d.reshape(cfg.NB, 128).T

    xT = np.zeros((NC, cfg.FIN, SP), dtype=np.float32)
    for c in range(NC):
        xc = np.zeros((SP, cfg.FIN), dtype=np.float32)
        xc[: cfg.SHARD] = x[perm_list[c]]
        xT[c] = np.ascontiguousarray(xc.T)

    return dict(offs=offs, dis_pm=dis_pm, xT=xT, S_list=S_list, NI=NI,
                perm_list=perm_list)


def _build_kernel(cfg, S_list, NI):
    nc = bacc.Bacc("TRN2")
    FIN, H, C, SP, NB = cfg.FIN, cfg.H, cfg.C, cfg.SHARD_PAD, cfg.NB
    KC, CH = cfg.KC, cfg.CHUNK

    xT = nc.dram_tensor("xT", [FIN, SP], F32, kind="ExternalInput")
    w1 = nc.dram_tensor("w1", [FIN, H], F32, kind="ExternalInput")
    b1r = nc.dram_tensor("b1r", [128, H], F32, kind="ExternalInput")
    w2 = nc.dram_tensor("w2", [H, C], F32, kind="ExternalInput")
    b2r = nc.dram_tensor("b2r", [128, C], F32, kind="ExternalInput")
    dis_d = nc.dram_tensor("dis", [128, NB], F32, kind="ExternalInput")
    offs_d = nc.dram_tensor("offs", [128, max(NI, 1)], I32, kind="ExternalInput")
    out_d = nc.dram_tensor("out", [SP, C], F32, kind="ExternalOutput")

    h1_own = nc.dram_tensor("h1_own", [SP, H], F32)
    y2_own = nc.dram_tensor("y2_own", [SP, H], F32)
    table1 = nc.dram_tensor("table1", [cfg.TROWS, H], F32, addr_space="Shared")
    table2 = nc.dram_tensor("table2", [cfg.TROWS, H], F32, addr_space="Shared")

    groups = [list(range(cfg.NC))]

    with TileContext(nc) as tc:
        with tc.tile_pool(name="const", bufs=1) as constp, \
             tc.tile_pool(name="xsup", bufs=2) as xsupp, \
             tc.tile_pool(name="ps_h", bufs=4, space="PSUM") as ps_h, \
             tc.tile_pool(name="ps_t", bufs=2, space="PSUM") as ps_t, \
             tc.tile_pool(name="ps_o", bufs=2, space="PSUM") as ps_o, \
             tc.tile_pool(name="hsb", bufs=4) as hsbp, \
             tc.tile_pool(name="acc", bufs=4) as accp, \
             tc.tile_pool(name="g", bufs=16) as gp, \
             tc.tile_pool(name="ep", bufs=4) as epp:

            w1t = constp.tile([CH, KC, H], F32)
            for k in range(KC):
                nc.sync.dma_start(out=w1t[:, k, :], in_=w1[k * CH:(k + 1) * CH, :])
            w2t = constp.tile([H, C], F32)
            nc.sync.dma_start(out=w2t[:], in_=w2[:])
            b1t = constp.tile([128, H], F32)
            nc.sync.dma_start(out=b1t[:], in_=b1r[:])
            b2t = constp.tile([128, C], F32)
            nc.sync.dma_start(out=b2t[:], in_=b2r[:])
            dis_t = constp.tile([128, NB], F32)
            nc.sync.dma_start(out=dis_t[:], in_=dis_d[:])
            offs_t = constp.tile([128, max(NI, 1)], I32)
            nc.sync.dma_start(out=offs_t[:], in_=offs_d[:])
            ident = constp.tile([128, 128], F32)
            make_identity(nc, ident[:])

            # Phase A: h1_own = dis * (x @ W1)
            nxs = (NB + cfg.XS - 1) // cfg.XS
            for si in range(nxs):
                b_lo = si * cfg.XS
                b_hi = min(NB, b_lo + cfg.XS)
                w = (b_hi - b_lo) * 128
                xts = xsupp.tile([CH, KC, cfg.XS * 128], F32, tag="xts")
                for k in range(KC):
                    nc.sync.dma_start(
                        out=xts[:, k, :w],
                        in_=xT[k * CH:(k + 1) * CH, b_lo * 128:b_hi * 128])
                for b in range(b_lo, b_hi):
                    j = (b - b_lo) * 128
                    ph = ps_h.tile([128, H], F32, tag="ph")
                    for k in range(KC):
                        nc.tensor.matmul(
                            out=ph[:], lhsT=xts[:, k, j:j + 128],
                            rhs=w1t[:, k, :],
                            start=(k == 0), stop=(k == KC - 1))
                    hsb = hsbp.tile([128, H], F32, tag="hsb")
                    nc.scalar.mul(out=hsb[:], in_=ph[:], mul=dis_t[:, b:b + 1])
                    nc.sync.dma_start(out=h1_own[b * 128:(b + 1) * 128, :],
                                      in_=hsb[:])

            # Phase B: AllGather h1 -> table1
            nc.gpsimd.collective_compute(
                "AllGather", mybir.AluOpType.bypass, replica_groups=groups,
                ins=[h1_own[:, :]], outs=[table1[:, :]])

            def aggregate(table, own, post_block):
                t = 0
                for b in range(NB):
                    acc = accp.tile([128, H], F32, tag="acc")
                    nc.sync.dma_start(
                        out=acc[:], in_=own[b * 128:(b + 1) * 128, :])
                    for _s in range(int(S_list[b])):
                        g = gp.tile([128, H], F32, tag="g")
                        nc.gpsimd.indirect_dma_start(
                            out=g[:], out_offset=None, in_=table[:, :],
                            in_offset=bass.IndirectOffsetOnAxis(
                                ap=offs_t[:, t:t + 1], axis=0))
                        nc.vector.tensor_add(out=acc[:], in0=acc[:], in1=g[:])
                        t += 1
                    post_block(b, acc)

            def post1(b, acc):
                dis_col = dis_t[:, b:b + 1]
                v = epp.tile([128, H], F32, tag="v1")
                nc.vector.tensor_scalar_mul(out=v[:], in0=acc[:], scalar1=dis_col)
                nc.vector.tensor_add(out=v[:], in0=v[:], in1=b1t[:])
                r = epp.tile([128, H], F32, tag="r1")
                nc.scalar.activation(out=r[:], in_=v[:],
                                     func=mybir.ActivationFunctionType.Relu)
                y = epp.tile([128, H], F32, tag="y1")
                nc.vector.tensor_scalar_mul(out=y[:], in0=r[:], scalar1=dis_col)
                nc.sync.dma_start(out=y2_own[b * 128:(b + 1) * 128, :], in_=y[:])

            aggregate(table1, h1_own, post1)

            # Phase D: AllGather y2 -> table2
            nc.gpsimd.collective_compute(
                "AllGather", mybir.AluOpType.bypass, replica_groups=groups,
                ins=[y2_own[:, :]], outs=[table2[:, :]])

            def post2(b, acc):
                dis_col = dis_t[:, b:b + 1]
                a = epp.tile([128, H], F32, tag="a2")
                nc.vector.tensor_scalar_mul(out=a[:], in0=acc[:], scalar1=dis_col)
                pt = ps_t.tile([H, 128], F32, tag="pt")
                nc.tensor.transpose(out=pt[:], in_=a[:], identity=ident[:])
                at = epp.tile([H, 128], F32, tag="at")
                nc.vector.tensor_copy(out=at[:], in_=pt[:])
                po = ps_o.tile([128, C], F32, tag="po")
                nc.tensor.matmul(out=po[:], lhsT=at[:], rhs=w2t[:],
                                 start=True, stop=True)
                o1 = epp.tile([128, C], F32, tag="o1")
                nc.vector.tensor_add(out=o1[:], in0=po[:], in1=b2t[:])
                mx = epp.tile([128, 1], F32, tag="mx")
                nc.vector.reduce_max(out=mx[:], in_=o1[:],
                                     axis=mybir.AxisListType.X)
                tt = epp.tile([128, C], F32, tag="tt")
                nc.vector.tensor_scalar(out=tt[:], in0=o1[:], scalar1=mx[:],
                                        scalar2=None,
                                        op0=mybir.AluOpType.subtract)
                ex = epp.tile([128, C], F32, tag="ex")
                nc.scalar.activation(out=ex[:], in_=tt[:],
                                     func=mybir.ActivationFunctionType.Exp)
                sm = epp.tile([128, 1], F32, tag="sm")
                nc.vector.reduce_sum(out=sm[:], in_=ex[:],
                                     axis=mybir.AxisListType.X)
                ls = epp.tile([128, 1], F32, tag="ls")
                nc.scalar.activation(out=ls[:], in_=sm[:],
                                     func=mybir.ActivationFunctionType.Ln)
                fin = epp.tile([128, C], F32, tag="fin")
                nc.vector.tensor_scalar(out=fin[:], in0=tt[:], scalar1=ls[:],
                                        scalar2=None,
                                        op0=mybir.AluOpType.subtract)
                nc.sync.dma_start(out=out_d[b * 128:(b + 1) * 128, :], in_=fin[:])

            aggregate(table2, y2_own, post2)

    nc.compile()
    return nc


def kernel(x, edge_index, W1, b1, W2, b2):
    x = np.asarray(x)
    edge_index = np.asarray(edge_index)
    W1 = np.asarray(W1, np.float32)
    b1 = np.asarray(b1, np.float32)
    W2 = np.asarray(W2, np.float32)
    b2 = np.asarray(b2, np.float32)

    cfg = _Cfg(x.shape[0], x.shape[1], W1.shape[1], W2.shape[1], N_CORES)
    pre = _preprocess(x, edge_index, cfg)
    nc = _build_kernel(cfg, pre["S_list"], pre["NI"])

    b1r = np.broadcast_to(b1, (128, cfg.H)).copy()
    b2r = np.broadcast_to(b2, (128, cfg.C)).copy()
    in_maps = []
    for c in range(cfg.NC):
        in_maps.append({
            "xT": pre["xT"][c],
            "w1": W1,
            "b1r": b1r,
            "w2": W2,
            "b2r": b2r,
            "dis": pre["dis_pm"][c],
            "offs": pre["offs"][c],
        })
    r = run_bass_kernel_spmd(nc, in_maps, list(range(cfg.NC)))
    out = np.empty((cfg.N, cfg.C), dtype=np.float32)
    for c in range(cfg.NC):
        out[pre["perm_list"][c]] = r.results[c]["out"][: cfg.SHARD]
    return out
